# revision 5
# baseline (speedup 1.0000x reference)
"""Trainium2 Bass kernel for nn_CustomLoss_43645457662200 (loss_fn).

Pure data-parallel over 8 NeuronCores; host combines per-core partial
sums (the "all-reduce of scalars").  Phased schedule: [l|u] DMAs are
front-loaded so DVE always has A-phase work while B/S blocks stream.

Math (all bf16, host casts only + integer mask prep s=(1-2pv)*(dt!=0)):
    su = l+u (DVE); c = 0.5*su acc Sc (DVE TS 4x); sl: acc Sl (DVE TS)
    d = u-l (Pool, off critical path) -> ACT Abs acc Sabs
    y = c-t (DVE) -> ACT Square acc Ssq
    g = c-p (DVE); q = g*s (DVE) -> relu acc Spen (DVE TS or ACT)
    Sd = 2(Sc - Sl)
    total = [1.5*Ssq + 0.1*Sd + 5*(Sabs-Sd) + 0.5*Spen]/B

The whole su->c->y/g->q->relu chain sits on the in-order DVE where it
packs with no cross-engine bubbles; Pool and ACT run the off-chain ops.
Stage accum columns are grouped per tile so the output flushes as two
contiguous DMAs (tiles 0..nt-2 early, last tile late).
"""

import numpy as np

from concourse import bass, mybir
from concourse.bass_utils import run_bass_kernel_spmd
from concourse.tile import TileContext


B = 4_194_304
NCORES = 8
N = B // NCORES
P = 128
CPT = N // P  # 4096

f32 = mybir.dt.float32
bf16 = mybir.dt.bfloat16

DEFAULT_CFG = dict(
    sizes=[704, 1280, 1280, 832],
    su_eng=["v", "v", "v", "v"],
    d_eng=["p", "p", "p", "p"],      # 'p' Pool, 'v' DVE
    sl_act=[False, False, False, False],
    relu_act=[False, False, False, False],  # True -> ACT Relu, False -> DVE TS
    abs_act=[True, True, True, True],       # True -> ACT Abs, False -> DVE TS
    split_dma=3,
    split_out=True,
    phased=False,
)


def _legalize_sync_waits(nc: bass.Bass) -> bass.Bass:
    counter = 0
    for fn in nc.m.functions:
        for blk in fn.blocks:
            insts = blk.instructions
            out = []
            changed = False
            for ins in insts:
                si = ins.sync_info
                waits = list(si.on_wait) if si is not None and si.on_wait else []
                if len(waits) > 1:
                    changed = True
                    for w in waits[:-1]:
                        counter += 1
                        nop = mybir.InstNoOp(name=f"waitsplit_{counter}")
                        nop.engine = ins.engine
                        nop.sync_info = mybir.SyncInfo(on_wait=[w], on_update=[])
                        out.append(nop)
                    ins.sync_info = mybir.SyncInfo(
                        on_wait=[waits[-1]], on_update=list(si.on_update or [])
                    )
                out.append(ins)
            if changed:
                blk.instructions = out
    return nc


def _phase2(nc, midpool, sqj, penj, tiles, c, stage, i, F):
    Op = mybir.AluOpType
    Act = mybir.ActivationFunctionType
    bfl = mybir.dt.bfloat16
    t = tiles[("B", i)][:, 0:F]
    p = tiles[("B", i)][:, F : 2 * F]
    s = tiles[("S", i)][:, 0:F]
    y = midpool.tile([128, F], bfl, tag=f"y{i}", name=f"y{i}")
    nc.vector.tensor_tensor(out=y, in0=c, in1=t, op=Op.subtract)
    g = midpool.tile([128, F], bfl, tag=f"g{i}", name=f"g{i}")
    nc.vector.tensor_tensor(out=g, in0=c, in1=p, op=Op.subtract)
    q = midpool.tile([128, F], bfl, tag=f"q{i}", name=f"q{i}")
    nc.vector.tensor_tensor(out=q, in0=g, in1=s, op=Op.mult)
    nc.scalar.activation(
        out=sqj[:, 0:F], in_=y, func=Act.Square,
        accum_out=stage[:, 5 * i + 3 : 5 * i + 4],
    )
    nc.vector.tensor_scalar(
        out=penj[:, 0:F], in0=q, scalar1=0.0, scalar2=None,
        op0=Op.max, op1=Op.add, accum_out=stage[:, 5 * i + 4 : 5 * i + 5],
    )


def build_program(cpt: int = CPT, cfg=None, legalize: bool = True) -> bass.Bass:
    if cfg is None:
        cfg = DEFAULT_CFG
    tile_sizes = cfg["sizes"] if cpt == CPT else [cpt]
    assert sum(tile_sizes) == cpt
    nt = len(tile_sizes)
    d_eng = cfg.get("d_eng", ["p"] * nt) if cpt == CPT else ["v"] * nt
    su_eng = cfg.get("su_eng", ["v"] * nt) if cpt == CPT else ["v"] * nt
    sl_act = cfg.get("sl_act", [False] * nt) if cpt == CPT else [False] * nt
    relu_act = cfg.get("relu_act", [False] * nt) if cpt == CPT else [False] * nt
    abs_act = cfg.get("abs_act", [True] * nt) if cpt == CPT else [True] * nt
    split_dma = cfg.get("split_dma", True)
    split_out = cfg.get("split_out", True)

    Op = mybir.AluOpType
    Act = mybir.ActivationFunctionType

    nc = bass.Bass()
    packed = nc.declare_dram_parameter("packed", [P, 5 * cpt], bf16, isOutput=False)
    # stage layout: per tile 5 adjacent columns [Sl, Sc, Sabs, Ssq, Spen]
    acc_out = nc.declare_dram_parameter("acc_out", [P, 5 * nt], f32, isOutput=True)

    with TileContext(nc) as tc:
        with (
            tc.tile_pool(name="accs", bufs=1) as accpool,
            tc.tile_pool(name="io", bufs=1) as iopool,
            tc.tile_pool(name="mid", bufs=1) as midpool,
            tc.tile_pool(name="junk", bufs=1) as junkpool,
        ):
            stage = accpool.tile([P, 5 * nt], f32, tag="stage")

            fmax = max(tile_sizes)
            slj = junkpool.tile([P, fmax], bf16, tag="slj")
            absj = junkpool.tile([P, fmax], bf16, tag="absj")
            sqj = junkpool.tile([P, fmax], bf16, tag="sqj")
            penj = junkpool.tile([P, fmax], bf16, tag="penj")

            if cfg.get("phased", False):
                # phase-restructured: front-load [l|u] DMAs, run A-dependent
                # ops as they land, then B/S-dependent ops per tile.
                cols = []
                off = 0
                for F in tile_sizes:
                    cols.append(off)
                    off += 5 * F
                pkAs, pkBs, pkSs = [], [], []
                # DMA issue order: A0 A1 B0 S0 A2 B1 S1 A3 B2 S2 B3 S3
                order = []
                for i in range(nt):
                    order.append(("A", i))
                    if i >= 1:
                        order.append(("B", i - 1))
                        order.append(("S", i - 1))
                order.append(("B", nt - 1))
                order.append(("S", nt - 1))
                tiles = {}
                for kind, i in order:
                    F = tile_sizes[i]
                    c0 = cols[i]
                    if kind == "A":
                        pk = iopool.tile([P, 2 * F], bf16, tag=f"pkA{i}", name=f"pkA{i}")
                        nc.sync.dma_start(out=pk, in_=packed[:, c0 : c0 + 2 * F])
                    elif kind == "B":
                        pk = iopool.tile([P, 2 * F], bf16, tag=f"pkB{i}", name=f"pkB{i}")
                        nc.sync.dma_start(
                            out=pk, in_=packed[:, c0 + 3 * F : c0 + 5 * F]
                        )
                    else:
                        pk = iopool.tile([P, F], bf16, tag=f"pkS{i}", name=f"pkS{i}")
                        nc.sync.dma_start(
                            out=pk, in_=packed[:, c0 + 2 * F : c0 + 3 * F]
                        )
                    tiles[(kind, i)] = pk
                sus, cs = [], []
                # phase 1 per tile: su, sl-acc, c, d, abs
                for i, F in enumerate(tile_sizes):
                    l = tiles[("A", i)][:, 0:F]
                    u = tiles[("A", i)][:, F : 2 * F]
                    su = midpool.tile([P, F], bf16, tag=f"su{i}", name=f"su{i}")
                    nc.vector.tensor_tensor(out=su, in0=l, in1=u, op=Op.add)
                    nc.vector.tensor_scalar(
                        out=slj[:, 0:F], in0=l, scalar1=1.0, scalar2=None,
                        op0=Op.mult, op1=Op.add,
                        accum_out=stage[:, 5 * i : 5 * i + 1],
                    )
                    c = midpool.tile([P, F], bf16, tag=f"c{i}", name=f"c{i}")
                    nc.vector.tensor_scalar(
                        out=c, in0=su, scalar1=0.5, scalar2=None, op0=Op.mult,
                        op1=Op.add, accum_out=stage[:, 5 * i + 1 : 5 * i + 2],
                    )
                    cs.append(c)
                    d = midpool.tile([P, F], bf16, tag=f"d{i}", name=f"d{i}")
                    nc.gpsimd.tensor_tensor(out=d, in0=u, in1=l, op=Op.subtract)
                    nc.scalar.activation(
                        out=absj[:, 0:F], in_=d, func=Act.Abs,
                        accum_out=stage[:, 5 * i + 2 : 5 * i + 3],
                    )
                    # phase 2 for the PREVIOUS tile (its B/S have landed)
                    j = i - 1
                    if j >= 0:
                        Fj = tile_sizes[j]
                        _phase2(nc, midpool, sqj, penj, tiles, cs[j], stage, j, Fj)
                _phase2(nc, midpool, sqj, penj, tiles, cs[nt - 1], stage,
                        nt - 1, tile_sizes[nt - 1])
                if split_out and nt > 1:
                    nc.sync.dma_start(
                        out=acc_out[:, 0 : 5 * (nt - 1)],
                        in_=stage[:, 0 : 5 * (nt - 1)],
                    )
                    nc.sync.dma_start(
                        out=acc_out[:, 5 * (nt - 1) : 5 * nt],
                        in_=stage[:, 5 * (nt - 1) : 5 * nt],
                    )
                else:
                    nc.sync.dma_start(out=acc_out[:, :], in_=stage)
                return _legalize_sync_waits(nc) if legalize else nc

            col = 0
            for i, F in enumerate(tile_sizes):
                sl_acc = stage[:, 5 * i : 5 * i + 1]
                c_acc = stage[:, 5 * i + 1 : 5 * i + 2]
                abs_acc = stage[:, 5 * i + 2 : 5 * i + 3]
                sq_acc = stage[:, 5 * i + 3 : 5 * i + 4]
                pen_acc = stage[:, 5 * i + 4 : 5 * i + 5]

                if split_dma == 3:
                    pkA = iopool.tile([P, 2 * F], bf16, tag=f"pkA{i}", name=f"pkA{i}")
                    nc.sync.dma_start(out=pkA, in_=packed[:, col : col + 2 * F])
                    pkB = iopool.tile([P, 2 * F], bf16, tag=f"pkB{i}", name=f"pkB{i}")
                    nc.sync.dma_start(
                        out=pkB, in_=packed[:, col + 3 * F : col + 5 * F]
                    )
                    pkS = iopool.tile([P, F], bf16, tag=f"pkS{i}", name=f"pkS{i}")
                    nc.sync.dma_start(
                        out=pkS, in_=packed[:, col + 2 * F : col + 3 * F]
                    )
                    l = pkA[:, 0:F]
                    u = pkA[:, F : 2 * F]
                    t = pkB[:, 0:F]
                    p = pkB[:, F : 2 * F]
                    s = pkS[:, 0:F]
                elif split_dma:
                    pkA = iopool.tile([P, 2 * F], bf16, tag=f"pkA{i}", name=f"pkA{i}")
                    nc.sync.dma_start(out=pkA, in_=packed[:, col : col + 2 * F])
                    pkB = iopool.tile([P, 3 * F], bf16, tag=f"pkB{i}", name=f"pkB{i}")
                    nc.sync.dma_start(
                        out=pkB, in_=packed[:, col + 2 * F : col + 5 * F]
                    )
                    l = pkA[:, 0:F]
                    u = pkA[:, F : 2 * F]
                    s = pkB[:, 0:F]
                    t = pkB[:, F : 2 * F]
                    p = pkB[:, 2 * F : 3 * F]
                else:
                    pk = iopool.tile([P, 5 * F], bf16, tag=f"pk{i}", name=f"pk{i}")
                    nc.sync.dma_start(out=pk, in_=packed[:, col : col + 5 * F])
                    l = pk[:, 0:F]
                    u = pk[:, F : 2 * F]
                    s = pk[:, 2 * F : 3 * F]
                    t = pk[:, 3 * F : 4 * F]
                    p = pk[:, 4 * F : 5 * F]
                col += 5 * F

                # DVE chain: su -> c (acc Sc); sl acc
                su = midpool.tile([P, F], bf16, tag=f"su{i}", name=f"su{i}")
                sueng = nc.gpsimd if su_eng[i] == "p" else nc.vector
                sueng.tensor_tensor(out=su, in0=l, in1=u, op=Op.add)
                if sl_act[i]:
                    nc.scalar.activation(
                        out=slj[:, 0:F], in_=l, func=Act.Copy, accum_out=sl_acc
                    )
                else:
                    nc.vector.tensor_scalar(
                        out=slj[:, 0:F], in0=l, scalar1=1.0, scalar2=None,
                        op0=Op.mult, op1=Op.add, accum_out=sl_acc,
                    )
                c = midpool.tile([P, F], bf16, tag=f"c{i}", name=f"c{i}")
                nc.vector.tensor_scalar(
                    out=c, in0=su, scalar1=0.5, scalar2=None, op0=Op.mult,
                    op1=Op.add, accum_out=c_acc,
                )

                # Pool (off-chain): d = u - l -> ACT Abs
                d = midpool.tile([P, F], bf16, tag=f"d{i}", name=f"d{i}")
                deng = nc.gpsimd if d_eng[i] == "p" else nc.vector
                deng.tensor_tensor(out=d, in0=u, in1=l, op=Op.subtract)
                if abs_act[i]:
                    nc.scalar.activation(
                        out=absj[:, 0:F], in_=d, func=Act.Abs, accum_out=abs_acc
                    )
                else:
                    nc.vector.tensor_scalar(
                        out=absj[:, 0:F], in0=d, scalar1=0.0, scalar2=None,
                        op0=Op.abs_max, op1=Op.add, accum_out=abs_acc,
                    )

                # DVE chain: y, g, q
                y = midpool.tile([P, F], bf16, tag=f"y{i}", name=f"y{i}")
                nc.vector.tensor_tensor(out=y, in0=c, in1=t, op=Op.subtract)
                g = midpool.tile([P, F], bf16, tag=f"g{i}", name=f"g{i}")
                nc.vector.tensor_tensor(out=g, in0=c, in1=p, op=Op.subtract)
                q = midpool.tile([P, F], bf16, tag=f"q{i}", name=f"q{i}")
                nc.vector.tensor_tensor(out=q, in0=g, in1=s, op=Op.mult)

                # ACT: Square(y); relu per cfg
                nc.scalar.activation(
                    out=sqj[:, 0:F], in_=y, func=Act.Square, accum_out=sq_acc
                )
                if relu_act[i]:
                    nc.scalar.activation(
                        out=penj[:, 0:F], in_=q, func=Act.Relu, accum_out=pen_acc
                    )
                else:
                    nc.vector.tensor_scalar(
                        out=penj[:, 0:F], in0=q, scalar1=0.0, scalar2=None,
                        op0=Op.max, op1=Op.add, accum_out=pen_acc,
                    )

            outq = nc.scalar if cfg.get("out_scalar", False) else nc.sync
            if split_out and nt > 1:
                outq.dma_start(
                    out=acc_out[:, 0 : 5 * (nt - 1)], in_=stage[:, 0 : 5 * (nt - 1)]
                )
                outq.dma_start(
                    out=acc_out[:, 5 * (nt - 1) : 5 * nt],
                    in_=stage[:, 5 * (nt - 1) : 5 * nt],
                )
            else:
                outq.dma_start(out=acc_out[:, :], in_=stage)

    return _legalize_sync_waits(nc) if legalize else nc


def pack_arrays(l2, u2, s2, t2, p2, sizes):
    blocks = []
    off = 0
    for sz in sizes:
        fs = slice(off, off + sz)
        for arr in (l2, u2, s2, t2, p2):
            blocks.append(arr[:, fs])
        off += sz
    return np.ascontiguousarray(np.concatenate(blocks, axis=1))


def make_in_maps(pred, target, prev_pci, delta_time, pv_values, sizes=None):
    import ml_dtypes

    bf = ml_dtypes.bfloat16
    if sizes is None:
        sizes = DEFAULT_CFG["sizes"]
    predb = np.asarray(pred, np.float32).astype(bf)
    tb = np.asarray(target, np.float32).astype(bf)
    pb = np.asarray(prev_pci, np.float32).astype(bf)
    dt = np.asarray(delta_time, np.int64)
    pvi = np.asarray(pv_values, np.int64)
    sb = ((1 - 2 * pvi) * (dt != 0)).astype(bf)
    in_maps = []
    for k in range(NCORES):
        sl = slice(k * N, (k + 1) * N)
        l2 = np.ascontiguousarray(predb[sl, 0]).reshape(P, CPT)
        u2 = np.ascontiguousarray(predb[sl, 1]).reshape(P, CPT)
        t2 = np.ascontiguousarray(tb[sl, 0]).reshape(P, CPT)
        p2 = np.ascontiguousarray(pb[sl, 0]).reshape(P, CPT)
        s2 = sb[sl].reshape(P, CPT)
        in_maps.append({"packed": pack_arrays(l2, u2, s2, t2, p2, sizes)})
    return in_maps


def combine_partials(accs, n_total: int) -> np.ndarray:
    sl = sc = sabs = ssq = spen = 0.0
    for acc in accs:
        a = np.asarray(acc, dtype=np.float64)
        nt = a.shape[1] // 5
        v = a.reshape(a.shape[0], nt, 5)
        sl += v[:, :, 0].sum()
        sc += v[:, :, 1].sum()
        sabs += v[:, :, 2].sum()
        ssq += v[:, :, 3].sum()
        spen += v[:, :, 4].sum()
    sd = 2.0 * (sc - sl)
    total = (1.5 * ssq + 0.1 * sd + 5.0 * (sabs - sd) + 0.5 * spen) / float(n_total)
    return np.array(total, dtype=np.float32)


_PROGRAM = None


def _get_program() -> bass.Bass:
    global _PROGRAM
    if _PROGRAM is None:
        _PROGRAM = build_program()
    return _PROGRAM


def run_on_hw(pred, target, prev_pci, delta_time, pv_values, **runner_kwargs):
    nc = _get_program()
    in_maps = make_in_maps(pred, target, prev_pci, delta_time, pv_values)
    res = run_bass_kernel_spmd(nc, in_maps, list(range(NCORES)), **runner_kwargs)
    accs = [r["acc_out"] for r in res.results]
    return combine_partials(accs, B), res


def kernel(pred, target, prev_pci, delta_time, pv_values) -> np.ndarray:
    pred = np.asarray(pred, dtype=np.float32)
    target = np.asarray(target, dtype=np.float32)
    prev_pci = np.asarray(prev_pci, dtype=np.float32)
    delta_time = np.asarray(delta_time, dtype=np.int32)
    pv_values = np.asarray(pv_values, dtype=np.int32)
    total, _ = run_on_hw(pred, target, prev_pci, delta_time, pv_values)
    return total
